# revision 39
# baseline (speedup 1.0000x reference)
"""MoE head (top-2 of 8 experts, GELU MLP, residual + LayerNorm) on 8 trn2
NeuronCores.

Strategy (expert-parallel, mixed precision):
  - Host: router (logits -> top-2 -> softmax) exactly as the reference
    computes it (fp32).  Tokens are gathered per expert into capacity-padded
    buffers (capacity = actual max expert load, nothing dropped).
  - Device (8 cores, SPMD, core e owns expert e):
      GEMM1 in bf16 (x, W1 rounded to bf16 -> ~1e-3 error), weights
      stationary, h = gelu(x @ W1 + b1) written by the scalar engine
      directly as fp8 e4m3.
      GEMM2 in fp8 e4m3 with DoubleRow perf mode (2 MACs/cell/cycle):
      stationary = hT token tiles (each LDWEIGHTS amortized over the two
      512-wide H chunks), moving = W2 (scaled by 64 into fp8 range).
      psum partition dim = tokens, so the top-2 combine weight is a
      per-partition tensor_scalar multiply, and y leaves token-major.
    Total quantization error ~1.6e-2 rel (gate 2e-2): GEMM1 error is
    negligible in bf16; GEMM2 pays fp8 noise on h and W2 only.
  - Host: scatter-add the two expert contributions per token, residual
    add + LayerNorm, reshape to [B, T, H].

Self-contained: hardcodes the nn_MoEHead problem shapes
(B=2, T=2048, H=1024, F=4096, E=8, top-2).
"""

import os
import sys
import types

import numpy as np
import ml_dtypes


def _ensure_axon_ntff_hook():
    """bass_utils' axon trace path does `from antenv.axon_hooks import ...`;
    the container's antenv stub lacks that submodule, which would make any
    BASS_TRACE=1 run crash.  Recreate it, wiring the ctypes NTFF profiler
    hook from trn_agent_boot when available."""
    if "antenv.axon_hooks" in sys.modules:
        return
    mod = types.ModuleType("antenv.axon_hooks")
    hook = None
    try:
        from trn_agent_boot.trn_boot import _ntff_profile_via_ctypes

        so = "/opt/axon/libaxon_pjrt.so"
        if os.path.exists(so):
            hook = _ntff_profile_via_ctypes(so)
    except Exception:
        hook = None
    mod._hook = hook
    mod.get_axon_ntff_profile_hook = lambda: mod._hook

    def _set(h):
        mod._hook = h

    mod.set_axon_ntff_profile_hook = _set
    sys.modules["antenv.axon_hooks"] = mod
    try:
        import antenv

        antenv.axon_hooks = mod
    except Exception:
        pass


_ensure_axon_ntff_hook()

import concourse.bass as bass  # noqa: E402
import concourse.tile as tile  # noqa: E402
from concourse import bacc, mybir  # noqa: E402
from concourse.bass_utils import run_bass_kernel_spmd  # noqa: E402

P = 128
H = 1024
F = 4096
E = 8
TOP_K = 2
LN_EPS = 1e-5
KO = H // P  # 8 k-tiles for GEMM1 (contraction over H)
FO = F // P  # 32 f-tiles
S_W2 = 64.0  # fp8 scale for W2 (std 1/64 -> ~1); folded into combine wgt

# GEMM2 mode: "f8dr" = fp8 e4m3 + DoubleRow (2x PE rate, ~1.6e-2 rel err)
#             "bf16" = plain bf16 (safe fallback, ~1.5e-3 rel err)
G2_MODE = os.environ.get("MOE_G2", "f8dr")
# GEMM1 fp8 k-pairs: m in {0,1,2}; the first 256*m rows of the H contraction
# run as fp8 DoubleRow (x*16, W1*64 quantized), the rest bf16.  All GEMM1
# operands carry the 1024x scale; the gelu activation rescales by 1/1024.
# m=1 saves ~10us and raises rel err to ~1.82e-2 (gate 2e-2).
G1_M = int(os.environ.get("MOE_G1A", "1"))
S_X, S_W1 = 16.0, 64.0

_kernel_cache: dict = {}


def _tok_blocks(C):
    """Split C tokens into GEMM1 moving blocks of <=512, ascending sizes
    with a 256-token first block: small first-chain DMA footprint so the
    PE starts right after DGE bring-up.  (A 512 first block measures the
    same: the ~3us early stall is set by W1 chunk arrival, not chain pace.)"""
    first = min(256, C)
    rest = C - first
    blocks = [(0, first)]
    off = first
    nb = -(-rest // 512) if rest else 0
    szs = sorted(
        16 * (rest // 16 // nb + (1 if i < (rest // 16) % nb else 0))
        for i in range(nb)
    )
    for sz in szs:
        blocks.append((off, sz))
        off += sz
    assert off == C, (blocks, C)
    return blocks


def _build_moe_kernel(C, g2_mode, g1_m):
    """One expert's FFN over C capacity-padded tokens.

    in : xT [H-256*m, C] bf16, w1 [H-256*m, F] bf16 (the bf16 part of the
         H contraction; scaled by S_X / S_W1 when m>0),
         xT8 [256*m, C] fp8, w18 [256*m, F] fp8 (the fp8 DoubleRow part),
         b1v [F] f32, w2 [F, H] (fp8e4 scaled by S_W2, or bf16),
         wgt [TT*128] f32 (combine weight, already divided by S_W2)
    out: y [TT*128, H] f32 = (gelu(x @ W1 + b1) @ w2) * wgt[:, None]
    """
    f32 = mybir.dt.float32
    bf16 = mybir.dt.bfloat16
    f8 = mybir.dt.float8e4
    w2dt = f8 if g2_mode == "f8dr" else bf16
    hdt = f8 if g2_mode == "f8dr" else bf16
    TT = -(-C // P)  # token tiles of 128 (last may be partial)
    CT = TT * P
    KB = KO - 2 * g1_m  # bf16 k-tiles in GEMM1
    HB = KB * P

    nc = bacc.Bacc(None, target_bir_lowering=False, debug=False)

    xT = nc.dram_tensor("xT", [HB, C], bf16, kind="ExternalInput")
    w1 = nc.dram_tensor("w1", [HB, F], bf16, kind="ExternalInput")
    if g1_m:
        xT8 = nc.dram_tensor("xT8", [2 * g1_m * P, C], f8, kind="ExternalInput")
        w18 = nc.dram_tensor("w18", [2 * g1_m * P, F], f8, kind="ExternalInput")
        xT8_r = xT8.rearrange("(ko p) c -> p ko c", p=P)
        w18_r = w18.rearrange("(ko p) f -> p ko f", p=P)
    b1v = nc.dram_tensor("b1v", [F], f32, kind="ExternalInput")
    w2 = nc.dram_tensor("w2", [F, H], w2dt, kind="ExternalInput")
    wgt = nc.dram_tensor("wgt", [CT], f32, kind="ExternalInput")
    y = nc.dram_tensor("y", [CT, H], f32, kind="ExternalOutput")

    xT_r = xT.rearrange("(ko p) c -> p ko c", p=P)  # [128, KB, C]
    w1_r = w1.rearrange("(ko p) f -> p ko f", p=P)  # [128, KB, F]
    w2_r = w2.rearrange("(fo p) h -> p fo h", p=P)  # [128, 32, H]
    b1_r = b1v.rearrange("(fo p) -> p fo", p=P)  # [128, 32]
    wgt_r = wgt.rearrange("(tt p) -> p tt", p=P)  # [128, TT]
    y_r = y.rearrange("(tt p) h -> p tt h", p=P)  # [128, TT, H]

    blocks = _tok_blocks(C)

    with tile.TileContext(nc) as tc:
        with (
            tc.tile_pool(name="singles", bufs=1) as singles,
            tc.tile_pool(name="yp", bufs=4) as yp,
            tc.tile_pool(name="ps1", bufs=4, space="PSUM") as ps1,
            tc.tile_pool(name="ps2", bufs=4, space="PSUM") as ps2,
        ):
            # ---- DMA emission order == priority ----
            w1_sb = singles.tile([P, KB, F], bf16, name="w1_sb")
            if g1_m:
                w18_sb = singles.tile([P, 2 * g1_m, F], f8, name="w18_sb")
                xT8_sb = singles.tile([P, 2 * g1_m, C], f8, name="xT8_sb")
            # small first chunks of W1 (ft 0/1) + the 256-token first block
            # start the PE right after DGE bring-up; the rest streams behind.
            w1_chunks = [(0, 256), (256, 256)] + [
                (c, 512) for c in range(512, F, 512)
            ]
            # PE warm-up: the HAM clock-gate holds the PE at 1.2GHz until
            # ~3.4us of sustained matmul activity.  The PE is instruction-
            # ready at ~6us but first operands only land ~12us (DGE bring-up
            # + transfers), so ~4us of throwaway matmuls on a memset tile
            # flip the clock to 2.4GHz just before real work arrives.
            # (Sizing matters: issue cadence is ~110ns/MM, so 36 MMs give
            # the ~4us of sustained busy the HAM window needs; 20 were too
            # few to flip it, 50 N=256 overshot into real work by +12us.)
            warm_sb = singles.tile([P, P], bf16, name="warm_sb")
            nc.vector.memset(warm_sb[:], 1.0)
            for _wi in range(44):
                wps = ps1.tile([P, 512], f32, name="psum")
                nc.tensor.matmul(
                    wps[:, :P], warm_sb[:], warm_sb[:], start=True, stop=True
                )

            # Each dma_start costs ~0.7us of issue time on its engine queue;
            # the sync queue carries only the startup-critical w1/w18/xT
            # stream (w18 interleaved in need-order so no chunk's issue
            # delays a w1 chunk), everything else issues from the otherwise
            # idle gpsimd SWDGE queue in parallel.
            b1_sb = singles.tile([P, FO], f32, name="b1_sb")
            nc.gpsimd.dma_start(out=b1_sb[:], in_=b1_r[:])
            c0, cw = w1_chunks[0]
            nc.sync.dma_start(w1_sb[:, :, c0 : c0 + cw], w1_r[:, :, c0 : c0 + cw])
            xT_sb = singles.tile([P, KB, C], bf16, name="xT_sb")
            off0, sz0 = blocks[0]
            nc.sync.dma_start(xT_sb[:, :, 0:sz0], xT_r[:, :, 0:sz0])
            if g1_m:
                # the small fp8 startup operands flow on the gpsimd SWDGE
                # queue, in parallel with the sync queue's bf16 stream (the
                # DMA ramp is descriptor-rate-limited, so a second queue
                # gets the chain-closing DoubleRow operands in ~2us earlier)
                nc.gpsimd.dma_start(out=xT8_sb[:, :, 0:sz0], in_=xT8_r[:, :, 0:sz0])
                nc.gpsimd.dma_start(out=w18_sb[:, :, 0:1024], in_=w18_r[:, :, 0:1024])
            for c0, cw in w1_chunks[1:]:
                if g1_m and c0 % 1024 == 0:
                    nc.sync.dma_start(
                        w18_sb[:, :, c0 : c0 + 1024], w18_r[:, :, c0 : c0 + 1024]
                    )
                nc.sync.dma_start(
                    w1_sb[:, :, c0 : c0 + cw], w1_r[:, :, c0 : c0 + cw]
                )
            for off, sz in blocks[1:]:
                nc.sync.dma_start(
                    xT_sb[:, :, off : off + sz], xT_r[:, :, off : off + sz]
                )
                if g1_m:
                    nc.sync.dma_start(
                        xT8_sb[:, :, off : off + sz], xT8_r[:, :, off : off + sz]
                    )
            # w2/wgt stay LAST on the sync queue: DMA engines drain queues
            # in order, so bulk GEMM2 weights must not compete with the
            # startup-critical w1/xT stream (4MB early w2 flow costs ~6us
            # of PE start delay).
            w2_sb = singles.tile([P, FO, H], w2dt, name="w2_sb")
            W2_CH = 8  # fo-tiles per chunk
            for f0 in range(0, FO, W2_CH):
                nc.sync.dma_start(
                    w2_sb[:, f0 : f0 + W2_CH, :], w2_r[:, f0 : f0 + W2_CH, :]
                )
            wgt_sb = singles.tile([P, TT], f32, name="wgt_sb")
            nc.sync.dma_start(wgt_sb[:], wgt_r[:])

            hT_sb = singles.tile([P, FO, C], hdt, name="hT_sb")

            g1_scale = 1.0 / (S_X * S_W1) if g1_m else 1.0

            def gemm1(bi):
                off, sz = blocks[bi]
                for ft in range(FO):
                    psum = ps1.tile([P, 512], f32)
                    for k in range(KB):
                        nc.tensor.matmul(
                            psum[:, :sz],
                            w1_sb[:, k, ft * P : (ft + 1) * P],
                            xT_sb[:, k, off : off + sz],
                            start=(k == 0),
                            stop=(k == KB - 1 and not g1_m),
                        )
                    for j in range(g1_m):
                        nc.tensor.matmul(
                            psum[:, :sz],
                            w18_sb[:, 2 * j : 2 * j + 2, ft * P : (ft + 1) * P],
                            xT8_sb[:, 2 * j : 2 * j + 2, off : off + sz],
                            start=False,
                            stop=(j == g1_m - 1),
                            perf_mode=mybir.MatmulPerfMode.DoubleRow,
                        )
                    nc.scalar.activation(
                        hT_sb[:, ft, off : off + sz],
                        psum[:, :sz],
                        mybir.ActivationFunctionType.Gelu,
                        bias=b1_sb[:, ft : ft + 1],
                        scale=g1_scale,
                    )

            def gemm2(tt, last=False):
                tsz = min(P, C - tt * P)
                pss = [ps2.tile([P, 512], f32, name="ps2") for _hc in range(2)]
                if g2_mode == "f8dr":
                    for j in range(FO // 2):
                        lhsT = hT_sb[:, 2 * j : 2 * j + 2, tt * P : tt * P + tsz]
                        for hc in range(2):
                            nc.tensor.matmul(
                                pss[hc][:tsz, :],
                                lhsT,
                                w2_sb[:, 2 * j : 2 * j + 2, hc * 512 : (hc + 1) * 512],
                                start=(j == 0),
                                stop=(j == FO // 2 - 1),
                                perf_mode=mybir.MatmulPerfMode.DoubleRow,
                            )
                else:
                    for j in range(FO):
                        lhsT = hT_sb[:, j, tt * P : tt * P + tsz]
                        for hc in range(2):
                            nc.tensor.matmul(
                                pss[hc][:tsz, :],
                                lhsT,
                                w2_sb[:, j, hc * 512 : (hc + 1) * 512],
                                start=(j == 0),
                                stop=(j == FO - 1),
                            )
                for hc in range(2):
                    yt = yp.tile([P, 512], f32, name="yt")
                    if last and hc == 1:
                        # final epilogue: run the second combine-weight mul
                        # on the scalar engine so it overlaps the first on
                        # the vector engine (shorter kernel tail)
                        nc.scalar.activation(
                            yt[:tsz, :],
                            pss[hc][:tsz, :],
                            mybir.ActivationFunctionType.Copy,
                            scale=wgt_sb[:tsz, tt : tt + 1],
                        )
                    else:
                        nc.vector.tensor_scalar_mul(
                            yt[:tsz, :], pss[hc][:tsz, :], wgt_sb[:tsz, tt : tt + 1]
                        )
                    nc.sync.dma_start(
                        y_r[:tsz, tt, hc * 512 : (hc + 1) * 512], yt[:tsz, :]
                    )

            # Pipeline: GEMM2 for a block's token tiles is emitted one block
            # behind GEMM1, so the scalar-engine gelu latency never stalls
            # the PE.
            done_tt = 0
            prev_hi = 0
            for bi in range(len(blocks)):
                gemm1(bi)
                # token tiles fully covered by blocks[0..bi-1]
                hi = prev_hi
                if bi > 0:
                    hi = (blocks[bi - 1][0] + blocks[bi - 1][1]) // P
                for tt in range(done_tt, hi):
                    gemm2(tt)
                done_tt = max(done_tt, hi)
                prev_hi = hi
            for tt in range(done_tt, TT):
                gemm2(tt, last=(tt == TT - 1))

    nc.compile()
    return nc


def _get_kernel(C, g2_mode, g1_m):
    key = (C, g2_mode, g1_m)
    if key not in _kernel_cache:
        _kernel_cache[key] = _build_moe_kernel(C, g2_mode, g1_m)
    return _kernel_cache[key]


def _route(x, router_w, router_b):
    """Replicates the reference router bit-for-bit up to fp32 matmul
    rounding: logits -> top-2 (ties to lower index) -> softmax."""
    logits = x @ router_w.T + router_b  # [N, E] fp32
    order = np.argsort(-logits, axis=-1, kind="stable")
    idx = order[:, :TOP_K]  # [N, 2]
    vals = np.take_along_axis(logits, idx, axis=-1)
    vmax = vals.max(axis=-1, keepdims=True)
    ex = np.exp(vals - vmax)
    w = ex / ex.sum(axis=-1, keepdims=True)
    return idx, w.astype(np.float32)


def kernel(
    hidden_states,
    router_w,
    router_b,
    W1,
    b1,
    W2,
    b2,
    ln_gamma,
    ln_beta,
):
    hidden_states = np.asarray(hidden_states, np.float32)
    router_w = np.asarray(router_w, np.float32)
    router_b = np.asarray(router_b, np.float32)
    W1 = np.asarray(W1, np.float32)
    b1 = np.asarray(b1, np.float32)
    W2 = np.asarray(W2, np.float32)
    b2 = np.asarray(b2, np.float32)
    ln_gamma = np.asarray(ln_gamma, np.float32)
    ln_beta = np.asarray(ln_beta, np.float32)

    B, T, Hdim = hidden_states.shape
    N = B * T
    x = np.ascontiguousarray(hidden_states.reshape(N, Hdim))

    idx, topw = _route(x, router_w, router_b)

    tok_ids = np.arange(N)
    toks_per_e = []
    wts_per_e = []
    for e in range(E):
        sel0 = idx[:, 0] == e
        sel1 = idx[:, 1] == e
        toks = np.concatenate([tok_ids[sel0], tok_ids[sel1]])
        ws = np.concatenate([topw[sel0, 0], topw[sel1, 1]])
        toks_per_e.append(toks)
        wts_per_e.append(ws)

    max_cnt = max(len(t) for t in toks_per_e)
    # capacity: multiple of 16 keeps DMA rows aligned; >=256 keeps the PE
    # at full rate
    C = max(((max_cnt + 15) // 16) * 16, 256)
    TT = -(-C // P)
    CT = TT * P

    nc = _get_kernel(C, G2_MODE, G1_M)

    bf16 = ml_dtypes.bfloat16
    f8 = ml_dtypes.float8_e4m3
    w2dt = f8 if G2_MODE == "f8dr" else bf16
    wscale = np.float32(S_W2) if G2_MODE == "f8dr" else np.float32(1.0)
    ks = 2 * G1_M * P  # H rows handled by the fp8 part of GEMM1
    g1s = np.float32(S_X) if G1_M else np.float32(1.0)
    g1ws = np.float32(S_W1) if G1_M else np.float32(1.0)

    in_maps = []
    for e in range(E):
        toks = toks_per_e[e]
        n = len(toks)
        X = np.zeros((C, Hdim), dtype=np.float32)
        X[:n] = x[toks]
        Xs = (X * g1s).T  # [H, C]
        wv = np.zeros((CT,), dtype=np.float32)
        wv[:n] = wts_per_e[e] / wscale
        w1s = W1[e] * g1ws if G1_M else W1[e]
        im = {
            "xT": np.ascontiguousarray(Xs[ks:]).astype(bf16),
            "w1": np.ascontiguousarray(w1s[ks:]).astype(bf16),
            "b1v": b1[e],
            "w2": (W2[e] * wscale).astype(w2dt),
            "wgt": wv,
        }
        if G1_M:
            im["xT8"] = np.ascontiguousarray(Xs[:ks]).astype(f8)
            im["w18"] = np.ascontiguousarray(w1s[:ks]).astype(f8)
        in_maps.append(im)

    res = run_bass_kernel_spmd(nc, in_maps, core_ids=list(range(E)))

    out = np.zeros((N, Hdim), dtype=np.float64)
    for e in range(E):
        toks = toks_per_e[e]
        n = len(toks)
        ye = res.results[e]["y"][:n]  # [n, H] fp32, already combine-weighted
        out[toks] += ye.astype(np.float64)
        if b2[e].any():
            out[toks] += np.outer(
                wts_per_e[e].astype(np.float64), b2[e].astype(np.float64)
            )

    # residual + LayerNorm (float64 internally; reference is fp32)
    out += x.astype(np.float64)
    mu = out.mean(axis=-1, keepdims=True)
    var = out.var(axis=-1, keepdims=True)
    out = (out - mu) / np.sqrt(var + LN_EPS)
    out = out * ln_gamma.astype(np.float64) + ln_beta.astype(np.float64)

    return out.astype(np.float32).reshape(B, T, Hdim)


# revision 40
# speedup vs baseline: 1.0216x; 1.0216x over previous
"""MoE head (top-2 of 8 experts, GELU MLP, residual + LayerNorm) on 8 trn2
NeuronCores.

Strategy (expert-parallel, mixed precision):
  - Host: router (logits -> top-2 -> softmax) exactly as the reference
    computes it (fp32).  Tokens are gathered per expert into capacity-padded
    buffers (capacity = actual max expert load, nothing dropped).
  - Device (8 cores, SPMD, core e owns expert e):
      GEMM1 in bf16 (x, W1 rounded to bf16 -> ~1e-3 error), weights
      stationary, h = gelu(x @ W1 + b1) written by the scalar engine
      directly as fp8 e4m3.
      GEMM2 in fp8 e4m3 with DoubleRow perf mode (2 MACs/cell/cycle):
      stationary = hT token tiles (each LDWEIGHTS amortized over the two
      512-wide H chunks), moving = W2 (scaled by 64 into fp8 range).
      psum partition dim = tokens, so the top-2 combine weight is a
      per-partition tensor_scalar multiply, and y leaves token-major.
    Total quantization error ~1.6e-2 rel (gate 2e-2): GEMM1 error is
    negligible in bf16; GEMM2 pays fp8 noise on h and W2 only.
  - Host: scatter-add the two expert contributions per token, residual
    add + LayerNorm, reshape to [B, T, H].

Self-contained: hardcodes the nn_MoEHead problem shapes
(B=2, T=2048, H=1024, F=4096, E=8, top-2).
"""

import os
import sys
import types

import numpy as np
import ml_dtypes


def _ensure_axon_ntff_hook():
    """bass_utils' axon trace path does `from antenv.axon_hooks import ...`;
    the container's antenv stub lacks that submodule, which would make any
    BASS_TRACE=1 run crash.  Recreate it, wiring the ctypes NTFF profiler
    hook from trn_agent_boot when available."""
    if "antenv.axon_hooks" in sys.modules:
        return
    mod = types.ModuleType("antenv.axon_hooks")
    hook = None
    try:
        from trn_agent_boot.trn_boot import _ntff_profile_via_ctypes

        so = "/opt/axon/libaxon_pjrt.so"
        if os.path.exists(so):
            hook = _ntff_profile_via_ctypes(so)
    except Exception:
        hook = None
    mod._hook = hook
    mod.get_axon_ntff_profile_hook = lambda: mod._hook

    def _set(h):
        mod._hook = h

    mod.set_axon_ntff_profile_hook = _set
    sys.modules["antenv.axon_hooks"] = mod
    try:
        import antenv

        antenv.axon_hooks = mod
    except Exception:
        pass


_ensure_axon_ntff_hook()

import concourse.bass as bass  # noqa: E402
import concourse.tile as tile  # noqa: E402
from concourse import bacc, mybir  # noqa: E402
from concourse.bass_utils import run_bass_kernel_spmd  # noqa: E402

P = 128
H = 1024
F = 4096
E = 8
TOP_K = 2
LN_EPS = 1e-5
KO = H // P  # 8 k-tiles for GEMM1 (contraction over H)
FO = F // P  # 32 f-tiles
S_W2 = 64.0  # fp8 scale for W2 (std 1/64 -> ~1); folded into combine wgt

# GEMM2 mode: "f8dr" = fp8 e4m3 + DoubleRow (2x PE rate, ~1.6e-2 rel err)
#             "bf16" = plain bf16 (safe fallback, ~1.5e-3 rel err)
G2_MODE = os.environ.get("MOE_G2", "f8dr")
# GEMM1 fp8 k-pairs: m in {0,1,2}; the first 256*m rows of the H contraction
# run as fp8 DoubleRow (x*16, W1*64 quantized), the rest bf16.  All GEMM1
# operands carry the 1024x scale; the gelu activation rescales by 1/1024.
# m=1 saves ~10us and raises rel err to ~1.82e-2 (gate 2e-2).
G1_M = int(os.environ.get("MOE_G1A", "1"))
S_X, S_W1 = 16.0, 64.0

_kernel_cache: dict = {}


def _tok_blocks(C):
    """Split C tokens into GEMM1 moving blocks of <=512, ascending sizes
    with a 256-token first block: small first-chain DMA footprint so the
    PE starts right after DGE bring-up.  (A 512 first block measures the
    same: the ~3us early stall is set by W1 chunk arrival, not chain pace.)"""
    first = min(256, C)
    rest = C - first
    blocks = [(0, first)]
    off = first
    nb = -(-rest // 512) if rest else 0
    szs = sorted(
        16 * (rest // 16 // nb + (1 if i < (rest // 16) % nb else 0))
        for i in range(nb)
    )
    for sz in szs:
        blocks.append((off, sz))
        off += sz
    assert off == C, (blocks, C)
    return blocks


def _build_moe_kernel(C, g2_mode, g1_m):
    """One expert's FFN over C capacity-padded tokens.

    in : xT [H-256*m, C] bf16, w1 [H-256*m, F] bf16 (the bf16 part of the
         H contraction; scaled by S_X / S_W1 when m>0),
         xT8 [256*m, C] fp8, w18 [256*m, F] fp8 (the fp8 DoubleRow part),
         b1v [F] f32, w2 [F, H] (fp8e4 scaled by S_W2, or bf16),
         wgt [TT*128] f32 (combine weight, already divided by S_W2)
    out: y [TT*128, H] f32 = (gelu(x @ W1 + b1) @ w2) * wgt[:, None]
    """
    f32 = mybir.dt.float32
    bf16 = mybir.dt.bfloat16
    f8 = mybir.dt.float8e4
    w2dt = f8 if g2_mode == "f8dr" else bf16
    hdt = f8 if g2_mode == "f8dr" else bf16
    TT = -(-C // P)  # token tiles of 128 (last may be partial)
    CT = TT * P
    KB = KO - 2 * g1_m  # bf16 k-tiles in GEMM1
    HB = KB * P

    nc = bacc.Bacc(None, target_bir_lowering=False, debug=False)

    xT = nc.dram_tensor("xT", [HB, C], bf16, kind="ExternalInput")
    w1 = nc.dram_tensor("w1", [HB, F], bf16, kind="ExternalInput")
    if g1_m:
        xT8 = nc.dram_tensor("xT8", [2 * g1_m * P, C], f8, kind="ExternalInput")
        w18 = nc.dram_tensor("w18", [2 * g1_m * P, F], f8, kind="ExternalInput")
        xT8_r = xT8.rearrange("(ko p) c -> p ko c", p=P)
        w18_r = w18.rearrange("(ko p) f -> p ko f", p=P)
    b1v = nc.dram_tensor("b1v", [F], f32, kind="ExternalInput")
    w2 = nc.dram_tensor("w2", [F, H], w2dt, kind="ExternalInput")
    wgt = nc.dram_tensor("wgt", [CT], f32, kind="ExternalInput")
    y = nc.dram_tensor("y", [CT, H], f32, kind="ExternalOutput")

    xT_r = xT.rearrange("(ko p) c -> p ko c", p=P)  # [128, KB, C]
    w1_r = w1.rearrange("(ko p) f -> p ko f", p=P)  # [128, KB, F]
    w2_r = w2.rearrange("(fo p) h -> p fo h", p=P)  # [128, 32, H]
    b1_r = b1v.rearrange("(fo p) -> p fo", p=P)  # [128, 32]
    wgt_r = wgt.rearrange("(tt p) -> p tt", p=P)  # [128, TT]
    y_r = y.rearrange("(tt p) h -> p tt h", p=P)  # [128, TT, H]

    blocks = _tok_blocks(C)

    with tile.TileContext(nc) as tc:
        with (
            tc.tile_pool(name="singles", bufs=1) as singles,
            tc.tile_pool(name="yp", bufs=4) as yp,
            tc.tile_pool(name="ps1", bufs=4, space="PSUM") as ps1,
            tc.tile_pool(name="ps2", bufs=4, space="PSUM") as ps2,
        ):
            # ---- DMA emission order == priority ----
            w1_sb = singles.tile([P, KB, F], bf16, name="w1_sb")
            if g1_m:
                w18_sb = singles.tile([P, 2 * g1_m, F], f8, name="w18_sb")
                xT8_sb = singles.tile([P, 2 * g1_m, C], f8, name="xT8_sb")
            # small first chunks of W1 (ft 0/1) + the 256-token first block
            # start the PE right after DGE bring-up; the rest streams behind.
            w1_chunks = [(0, 256), (256, 256)] + [
                (c, 512) for c in range(512, F, 512)
            ]
            # PE warm-up: the HAM clock-gate holds the PE at 1.2GHz until
            # ~3.4us of sustained matmul activity.  The PE is instruction-
            # ready at ~6us but first operands only land ~12us (DGE bring-up
            # + transfers), so ~4us of throwaway matmuls on a memset tile
            # flip the clock to 2.4GHz just before real work arrives.
            # (Sizing matters: issue cadence is ~110ns/MM, so 36 MMs give
            # the ~4us of sustained busy the HAM window needs; 20 were too
            # few to flip it, 50 N=256 overshot into real work by +12us.)
            warm_sb = singles.tile([P, P], bf16, name="warm_sb")
            nc.vector.memset(warm_sb[:], 1.0)
            for _wi in range(44):
                wps = ps1.tile([P, 512], f32, name="psum")
                nc.tensor.matmul(
                    wps[:, :P], warm_sb[:], warm_sb[:], start=True, stop=True
                )

            # Each dma_start costs ~0.7us of issue time on its engine queue;
            # the sync queue carries only the startup-critical w1/w18/xT
            # stream (w18 interleaved in need-order so no chunk's issue
            # delays a w1 chunk), everything else issues from the otherwise
            # idle gpsimd SWDGE queue in parallel.
            b1_sb = singles.tile([P, FO], f32, name="b1_sb")
            nc.gpsimd.dma_start(out=b1_sb[:], in_=b1_r[:])
            c0, cw = w1_chunks[0]
            nc.sync.dma_start(w1_sb[:, :, c0 : c0 + cw], w1_r[:, :, c0 : c0 + cw])
            xT_sb = singles.tile([P, KB, C], bf16, name="xT_sb")
            off0, sz0 = blocks[0]
            nc.sync.dma_start(xT_sb[:, :, 0:sz0], xT_r[:, :, 0:sz0])
            if g1_m:
                # the fp8 operands come AFTER the first bf16 pair: chains
                # open on their bf16 matmuls, so the DoubleRow matmul (now
                # last in each chain) tolerates the later w18/xT8 arrival
                nc.sync.dma_start(xT8_sb[:, :, 0:sz0], xT8_r[:, :, 0:sz0])
                nc.sync.dma_start(w18_sb[:, :, 0:1024], w18_r[:, :, 0:1024])
            for c0, cw in w1_chunks[1:]:
                if g1_m and c0 % 1024 == 0:
                    nc.sync.dma_start(
                        w18_sb[:, :, c0 : c0 + 1024], w18_r[:, :, c0 : c0 + 1024]
                    )
                nc.sync.dma_start(
                    w1_sb[:, :, c0 : c0 + cw], w1_r[:, :, c0 : c0 + cw]
                )
            for off, sz in blocks[1:]:
                nc.sync.dma_start(
                    xT_sb[:, :, off : off + sz], xT_r[:, :, off : off + sz]
                )
                if g1_m:
                    nc.sync.dma_start(
                        xT8_sb[:, :, off : off + sz], xT8_r[:, :, off : off + sz]
                    )
            # w2/wgt stay LAST on the sync queue: DMA engines drain queues
            # in order, so bulk GEMM2 weights must not compete with the
            # startup-critical w1/xT stream (4MB early w2 flow costs ~6us
            # of PE start delay).
            w2_sb = singles.tile([P, FO, H], w2dt, name="w2_sb")
            W2_CH = 8  # fo-tiles per chunk
            for f0 in range(0, FO, W2_CH):
                nc.sync.dma_start(
                    w2_sb[:, f0 : f0 + W2_CH, :], w2_r[:, f0 : f0 + W2_CH, :]
                )
            wgt_sb = singles.tile([P, TT], f32, name="wgt_sb")
            nc.sync.dma_start(wgt_sb[:], wgt_r[:])

            hT_sb = singles.tile([P, FO, C], hdt, name="hT_sb")

            g1_scale = 1.0 / (S_X * S_W1) if g1_m else 1.0

            def gemm1(bi):
                off, sz = blocks[bi]
                for ft in range(FO):
                    psum = ps1.tile([P, 512], f32)
                    for k in range(KB):
                        nc.tensor.matmul(
                            psum[:, :sz],
                            w1_sb[:, k, ft * P : (ft + 1) * P],
                            xT_sb[:, k, off : off + sz],
                            start=(k == 0),
                            stop=(k == KB - 1 and not g1_m),
                        )
                    for j in range(g1_m):
                        nc.tensor.matmul(
                            psum[:, :sz],
                            w18_sb[:, 2 * j : 2 * j + 2, ft * P : (ft + 1) * P],
                            xT8_sb[:, 2 * j : 2 * j + 2, off : off + sz],
                            start=False,
                            stop=(j == g1_m - 1),
                            perf_mode=mybir.MatmulPerfMode.DoubleRow,
                        )
                    nc.scalar.activation(
                        hT_sb[:, ft, off : off + sz],
                        psum[:, :sz],
                        mybir.ActivationFunctionType.Gelu,
                        bias=b1_sb[:, ft : ft + 1],
                        scale=g1_scale,
                    )

            def gemm2(tt, last=False):
                tsz = min(P, C - tt * P)
                pss = [ps2.tile([P, 512], f32, name="ps2") for _hc in range(2)]
                if g2_mode == "f8dr":
                    for j in range(FO // 2):
                        lhsT = hT_sb[:, 2 * j : 2 * j + 2, tt * P : tt * P + tsz]
                        for hc in range(2):
                            nc.tensor.matmul(
                                pss[hc][:tsz, :],
                                lhsT,
                                w2_sb[:, 2 * j : 2 * j + 2, hc * 512 : (hc + 1) * 512],
                                start=(j == 0),
                                stop=(j == FO // 2 - 1),
                                perf_mode=mybir.MatmulPerfMode.DoubleRow,
                            )
                else:
                    for j in range(FO):
                        lhsT = hT_sb[:, j, tt * P : tt * P + tsz]
                        for hc in range(2):
                            nc.tensor.matmul(
                                pss[hc][:tsz, :],
                                lhsT,
                                w2_sb[:, j, hc * 512 : (hc + 1) * 512],
                                start=(j == 0),
                                stop=(j == FO - 1),
                            )
                for hc in range(2):
                    yt = yp.tile([P, 512], f32, name="yt")
                    if last and hc == 1:
                        # final epilogue: run the second combine-weight mul
                        # on the scalar engine so it overlaps the first on
                        # the vector engine (shorter kernel tail)
                        nc.scalar.activation(
                            yt[:tsz, :],
                            pss[hc][:tsz, :],
                            mybir.ActivationFunctionType.Copy,
                            scale=wgt_sb[:tsz, tt : tt + 1],
                        )
                    else:
                        nc.vector.tensor_scalar_mul(
                            yt[:tsz, :], pss[hc][:tsz, :], wgt_sb[:tsz, tt : tt + 1]
                        )
                    nc.sync.dma_start(
                        y_r[:tsz, tt, hc * 512 : (hc + 1) * 512], yt[:tsz, :]
                    )

            # Pipeline: GEMM2 for a block's token tiles is emitted one block
            # behind GEMM1, so the scalar-engine gelu latency never stalls
            # the PE.
            done_tt = 0
            prev_hi = 0
            for bi in range(len(blocks)):
                gemm1(bi)
                # token tiles fully covered by blocks[0..bi-1]
                hi = prev_hi
                if bi > 0:
                    hi = (blocks[bi - 1][0] + blocks[bi - 1][1]) // P
                for tt in range(done_tt, hi):
                    gemm2(tt)
                done_tt = max(done_tt, hi)
                prev_hi = hi
            for tt in range(done_tt, TT):
                gemm2(tt, last=(tt == TT - 1))

    nc.compile()
    return nc


def _get_kernel(C, g2_mode, g1_m):
    key = (C, g2_mode, g1_m)
    if key not in _kernel_cache:
        _kernel_cache[key] = _build_moe_kernel(C, g2_mode, g1_m)
    return _kernel_cache[key]


def _route(x, router_w, router_b):
    """Replicates the reference router bit-for-bit up to fp32 matmul
    rounding: logits -> top-2 (ties to lower index) -> softmax."""
    logits = x @ router_w.T + router_b  # [N, E] fp32
    order = np.argsort(-logits, axis=-1, kind="stable")
    idx = order[:, :TOP_K]  # [N, 2]
    vals = np.take_along_axis(logits, idx, axis=-1)
    vmax = vals.max(axis=-1, keepdims=True)
    ex = np.exp(vals - vmax)
    w = ex / ex.sum(axis=-1, keepdims=True)
    return idx, w.astype(np.float32)


def kernel(
    hidden_states,
    router_w,
    router_b,
    W1,
    b1,
    W2,
    b2,
    ln_gamma,
    ln_beta,
):
    hidden_states = np.asarray(hidden_states, np.float32)
    router_w = np.asarray(router_w, np.float32)
    router_b = np.asarray(router_b, np.float32)
    W1 = np.asarray(W1, np.float32)
    b1 = np.asarray(b1, np.float32)
    W2 = np.asarray(W2, np.float32)
    b2 = np.asarray(b2, np.float32)
    ln_gamma = np.asarray(ln_gamma, np.float32)
    ln_beta = np.asarray(ln_beta, np.float32)

    B, T, Hdim = hidden_states.shape
    N = B * T
    x = np.ascontiguousarray(hidden_states.reshape(N, Hdim))

    idx, topw = _route(x, router_w, router_b)

    tok_ids = np.arange(N)
    toks_per_e = []
    wts_per_e = []
    for e in range(E):
        sel0 = idx[:, 0] == e
        sel1 = idx[:, 1] == e
        toks = np.concatenate([tok_ids[sel0], tok_ids[sel1]])
        ws = np.concatenate([topw[sel0, 0], topw[sel1, 1]])
        toks_per_e.append(toks)
        wts_per_e.append(ws)

    max_cnt = max(len(t) for t in toks_per_e)
    # capacity: multiple of 16 keeps DMA rows aligned; >=256 keeps the PE
    # at full rate
    C = max(((max_cnt + 15) // 16) * 16, 256)
    TT = -(-C // P)
    CT = TT * P

    nc = _get_kernel(C, G2_MODE, G1_M)

    bf16 = ml_dtypes.bfloat16
    f8 = ml_dtypes.float8_e4m3
    w2dt = f8 if G2_MODE == "f8dr" else bf16
    wscale = np.float32(S_W2) if G2_MODE == "f8dr" else np.float32(1.0)
    ks = 2 * G1_M * P  # H rows handled by the fp8 part of GEMM1
    g1s = np.float32(S_X) if G1_M else np.float32(1.0)
    g1ws = np.float32(S_W1) if G1_M else np.float32(1.0)

    in_maps = []
    for e in range(E):
        toks = toks_per_e[e]
        n = len(toks)
        X = np.zeros((C, Hdim), dtype=np.float32)
        X[:n] = x[toks]
        Xs = (X * g1s).T  # [H, C]
        wv = np.zeros((CT,), dtype=np.float32)
        wv[:n] = wts_per_e[e] / wscale
        w1s = W1[e] * g1ws if G1_M else W1[e]
        im = {
            "xT": np.ascontiguousarray(Xs[ks:]).astype(bf16),
            "w1": np.ascontiguousarray(w1s[ks:]).astype(bf16),
            "b1v": b1[e],
            "w2": (W2[e] * wscale).astype(w2dt),
            "wgt": wv,
        }
        if G1_M:
            im["xT8"] = np.ascontiguousarray(Xs[:ks]).astype(f8)
            im["w18"] = np.ascontiguousarray(w1s[:ks]).astype(f8)
        in_maps.append(im)

    res = run_bass_kernel_spmd(nc, in_maps, core_ids=list(range(E)))

    out = np.zeros((N, Hdim), dtype=np.float64)
    for e in range(E):
        toks = toks_per_e[e]
        n = len(toks)
        ye = res.results[e]["y"][:n]  # [n, H] fp32, already combine-weighted
        out[toks] += ye.astype(np.float64)
        if b2[e].any():
            out[toks] += np.outer(
                wts_per_e[e].astype(np.float64), b2[e].astype(np.float64)
            )

    # residual + LayerNorm (float64 internally; reference is fp32)
    out += x.astype(np.float64)
    mu = out.mean(axis=-1, keepdims=True)
    var = out.var(axis=-1, keepdims=True)
    out = (out - mu) / np.sqrt(var + LN_EPS)
    out = out * ln_gamma.astype(np.float64) + ln_beta.astype(np.float64)

    return out.astype(np.float32).reshape(B, T, Hdim)
